# revision 18
# baseline (speedup 1.0000x reference)
"""ChebNet (magnetic-Laplacian ChebConv, K=2, 2 layers + linear classifier +
log_softmax) on 8 Trainium2 NeuronCores.

Strategy: 1D row-shard of the (dense) conjugated magnetic Laplacian Lc across
8 cores (512 rows each).  The Laplacian is assembled on host from the edge
list; all matmuls, Chebyshev recursion, biases, classifier and log_softmax
run on device.

Pipelined-boundary version: every product is split into 4 sub-products
(feature-half x node-half) with separate PSUM banks so the node-half-A
output can be evicted and AllGather'ed while node-half-B is still being
computed.  Consumers process gathered chunks in delivery order and close
their own A half first, so each boundary's round-0 collective overlaps
compute on both sides.  The Karatsuba sum panels (L_r+L_i and X_r+X_i) are
computed on device (DVE) instead of being shipped from HBM, and the L
panels are packed node-half-major so half A streams from HBM first.
"""

import sys

for _p in ("/opt/trn_rl_repo",):
    if _p not in sys.path:
        sys.path.insert(0, _p)

import numpy as np
import ml_dtypes

import concourse.bass as bass
import concourse.mybir as mybir
import concourse.tile as tile
from concourse import bacc
from concourse import bass_utils
from concourse.masks import make_identity

P = 128          # partitions
F = 256          # feature width of X / hidden layers
FH = F // P      # feature halves (2)
NK = 3           # Chebyshev orders (K+1)
C = 40           # classes
N_NODES = 4096
N_CORES = 8
TWO_PI = 2.0 * np.pi

f32 = mybir.dt.float32
f32r = mybir.dt.float32r
bf16 = mybir.dt.bfloat16


# ---------------------------------------------------------------------------
# Device program
# ---------------------------------------------------------------------------

def build_nc(n_nodes=N_NODES, n_cores=N_CORES):
    KC = n_nodes // P            # contraction chunks (32)
    SH = n_nodes // n_cores      # local rows per core (512)
    HH = SH // 2                 # node half (256)
    MT = SH // P                 # local row tiles (4)
    NT = HH // P                 # row tiles per half (2)
    LB = 4                       # L chunks per load DMA group

    nc = bacc.Bacc("TRN2", target_bir_lowering=False, debug=False,
                   num_devices=n_cores)

    din = {}
    for nm, shp, dt in [
        # L^T panels, node-half-major: [half][kc][HH]
        ("ltr", [P, KC * SH], bf16),
        ("lti", [P, KC * SH], bf16),
        # stationary X, node-major chunks, r/i interleaved per chunk
        ("xri", [P, KC * 2 * F], bf16),
        ("x0tr", [P, FH * SH], f32r), ("x0ti", [P, FH * SH], f32r),
        ("w1", [P, FH * NK * FH * P], f32r), ("w2", [P, FH * NK * FH * P], f32r),
        ("wc", [P, 2 * FH * P], f32r),
        ("b1", [P, FH], f32), ("b2", [P, FH], f32), ("bc", [P, 1], f32),
    ]:
        din[nm] = nc.dram_tensor(nm, shp, dt, kind="ExternalInput").ap()
    out_d = nc.dram_tensor("out", [SH, C], f32, kind="ExternalOutput").ap()

    with tile.TileContext(nc) as tc:
        with (
            tc.tile_pool(name="const", bufs=1) as const,
            tc.tile_pool(name="lres", bufs=1) as lres,
            tc.tile_pool(name="stat", bufs=1) as stat,
            tc.tile_pool(name="ftp", bufs=1) as ftp,
            tc.tile_pool(name="stg", bufs=1) as stg,
            tc.tile_pool(name="sm", bufs=2) as sm,
            tc.tile_pool(name="ps", bufs=1, space="PSUM") as ps,
            tc.tile_pool(name="dram", bufs=1, space="DRAM") as dram,
        ):
            # ---- resident Laplacian panels (ltr/lti from HBM once; the sum
            # panel is computed on DVE) ---------------------------------------
            ltr_sb = lres.tile([P, KC * SH], bf16, tag="ltr", bufs=1, name="ltr_sb")
            lti_sb = lres.tile([P, KC * SH], bf16, tag="lti", bufs=1, name="lti_sb")
            lts_sb = lres.tile([P, KC * SH], bf16, tag="lts", bufs=1, name="lts_sb")

            def load_l_half(half, k0, k1, add=True):
                """Load ltr/lti chunks [k0,k1) of node-half `half`; optionally
                compute the Karatsuba sum panel for the same span."""
                sl = slice(half * KC * HH + k0 * HH, half * KC * HH + k1 * HH)
                nc.sync.dma_start(ltr_sb[:, sl], din["ltr"][:, sl])
                nc.sync.dma_start(lti_sb[:, sl], din["lti"][:, sl])
                if add:
                    lts_add(half, k0, k1)

            def lts_add(half, k0, k1):
                sl = slice(half * KC * HH + k0 * HH, half * KC * HH + k1 * HH)
                nc.vector.tensor_add(lts_sb[:, sl], ltr_sb[:, sl], lti_sb[:, sl])

            # ---- identity (no HBM traffic; needed by first boundary) -------
            ident_f = const.tile([P, P], f32)
            make_identity(nc, ident_f[:])
            ident = const.tile([P, P], f32r)
            nc.vector.tensor_copy(ident[:], ident_f[:])

            # ---- collective-stream warmup: a tiny AllGather issued at the
            # very start absorbs the one-time stream init + rendezvous cost
            # (~10us trigger delay + barrier) inside product 1's shadow, so
            # the first real boundary collective starts promptly.
            warm_in = dram.tile([8, 8], f32, tag="warmin", bufs=1,
                                name="warm_in")
            warm_out = dram.tile([n_cores * 8, 8], f32, tag="warmout",
                                 bufs=1, name="warm_out", addr_space="Shared")
            nc.sync.dma_start(warm_in[:], ident_f[0:8, 0:8])
            nc.gpsimd.collective_compute(
                "AllGather", mybir.AluOpType.bypass,
                replica_groups=[list(range(n_cores))],
                ins=[warm_in.opt()], outs=[warm_out.opt()])

            # ---- helpers ---------------------------------------------------
            def alloc_stationary(idx):
                sri = stat.tile([P, KC * 2 * F], bf16, tag="sri", bufs=1,
                                name=f"sri{idx}")
                ssum = stat.tile([P, KC * F], bf16, tag="ssum", bufs=1,
                                 name=f"ssum{idx}")
                return sri, ssum

            def ssum_add(stats, k0, k1):
                """ssum[k0:k1) = r + i from the interleaved sri layout."""
                sri, ssum = stats
                sv = sri.rearrange("p (k f) -> p k f", f=2 * F)
                dv = ssum.rearrange("p (k f) -> p k f", f=F)
                nc.vector.tensor_add(dv[:, k0:k1], sv[:, k0:k1, 0:F],
                                     sv[:, k0:k1, F:2 * F])

            def sub_product(stats, idx, h, half, order, evict, pre_mm=None):
                """One (feature-half h, node-half half) Karatsuba sub-product:
                P1 = Lr@Sr, P2 = Li@Si, P3 = (Lr+Li)@(Sr+Si) over `order`
                chunks, each [P, HH] in its own PSUM bank.  evict(p1,p2,p3)
                combines them into the destination half."""
                sri, ssum = stats
                nm = f"{idx}_{h}_{half}"
                p1 = ps.tile([P, HH], f32, tag="prod", bufs=6, name=f"p1_{nm}")
                p2 = ps.tile([P, HH], f32, tag="prod", bufs=6, name=f"p2_{nm}")
                p3 = ps.tile([P, HH], f32, tag="prod", bufs=6, name=f"p3_{nm}")
                base = half * KC * HH
                for j, kc in enumerate(order):
                    if pre_mm is not None:
                        pre_mm(kc)
                    lr = ltr_sb[:, base + kc * HH: base + (kc + 1) * HH]
                    li = lti_sb[:, base + kc * HH: base + (kc + 1) * HH]
                    ls = lts_sb[:, base + kc * HH: base + (kc + 1) * HH]
                    o_r = kc * 2 * F + h * P
                    o_i = kc * 2 * F + F + h * P
                    o_s = kc * F + h * P
                    first, last = j == 0, j == len(order) - 1
                    nc.tensor.matmul(p1[:], lhsT=sri[:, o_r:o_r + P], rhs=lr,
                                     start=first, stop=last)
                    nc.tensor.matmul(p2[:], lhsT=sri[:, o_i:o_i + P], rhs=li,
                                     start=first, stop=last)
                    nc.tensor.matmul(p3[:], lhsT=ssum[:, o_s:o_s + P], rhs=ls,
                                     start=first, stop=last)
                evict(p1, p2, p3, h, half)

            # DVE may read at most ONE PSUM operand per op: bounce P2
            # through SBUF scratch, then combine against P1/P3.
            def evict_copy(dst_r, dst_i):
                def fn(p1, p2, p3, h, half):
                    sl = slice(h * SH + half * HH, h * SH + (half + 1) * HH)
                    with tc.high_priority():
                        t2 = stg.tile([P, HH], f32, tag="scr", bufs=2,
                                      name=f"t2c{id(dst_r)}_{h}_{half}")
                        nc.vector.tensor_copy(t2[:], p2[:])
                        nc.vector.tensor_sub(dst_r[:, sl], p1[:], t2[:])
                        nc.vector.tensor_sub(dst_i[:, sl], p3[:], t2[:])
                        nc.vector.tensor_sub(dst_i[:, sl], dst_i[:, sl], p1[:])
                return fn

            def evict_cheb(dst_r, dst_i, z0_r, z0_i):
                """dst = 2*Z - z0 (Chebyshev T2 step), fused eviction."""
                def fn(p1, p2, p3, h, half):
                    sl = slice(h * SH + half * HH, h * SH + (half + 1) * HH)
                    with tc.high_priority():
                        t2 = stg.tile([P, HH], f32, tag="scr", bufs=2,
                                      name=f"t2x{id(dst_r)}_{h}_{half}")
                        u = stg.tile([P, HH], f32, tag="scr2", bufs=2,
                                     name=f"u{id(dst_r)}_{h}_{half}")
                        nc.vector.tensor_copy(t2[:], p2[:])
                        nc.vector.tensor_sub(u[:], p1[:], t2[:])
                        nc.vector.scalar_tensor_tensor(
                            dst_r[:, sl], u[:], 2.0, z0_r[:, sl],
                            op0=mybir.AluOpType.mult,
                            op1=mybir.AluOpType.subtract)
                        nc.vector.tensor_sub(u[:], p3[:], t2[:])
                        nc.vector.tensor_sub(u[:], u[:], p1[:])
                        nc.vector.scalar_tensor_tensor(
                            dst_i[:, sl], u[:], 2.0, z0_i[:, sl],
                            op0=mybir.AluOpType.mult,
                            op1=mybir.AluOpType.subtract)
                return fn

            def fire_round(src_r, src_i, idx, ri, stats, order):
                """AG one node-half round: PE-transpose the half's local Z^T
                to node-major bf16, sub-AllGather, reload the delivered global
                chunks into `stats`, extend `order` with the delivered chunk
                ids."""
                t0 = ri * NT
                with tc.high_priority():
                    stage = stg.tile([P, MT * 2 * F], bf16, tag="stage", bufs=1,
                                     name=f"stage{idx}")
                    for mt in range(t0, t0 + NT):
                        for ci, src in enumerate((src_r, src_i)):
                            for h in range(FH):
                                tp = ps.tile([P, P], f32r, tag="aux", bufs=2,
                                             name=f"tp{idx}_{mt}_{ci}_{h}")
                                nc.tensor.transpose(
                                    tp[:],
                                    src[:, h * SH + mt * P:
                                        h * SH + (mt + 1) * P],
                                    ident[:])
                                dst = stage[:, mt * 2 * F + ci * F + h * P:
                                            mt * 2 * F + ci * F + (h + 1) * P]
                                nc.vector.tensor_copy(dst, tp[:])
                    cc_in = dram.tile([NT * P, 2 * F], bf16, tag=f"ccin{ri}",
                                      bufs=2, name=f"ccin{idx}_{ri}")
                    cc_out = dram.tile([n_cores * NT * P, 2 * F], bf16,
                                       tag=f"ccout{ri}", bufs=2,
                                       name=f"ccout{idx}_{ri}",
                                       addr_space="Shared")
                    nc.sync.dma_start(
                        cc_in.rearrange("(t p) f -> p t f", p=P),
                        stage.rearrange("p (mt f) -> p mt f", mt=MT)
                             [:, t0:t0 + NT])
                    nc.gpsimd.collective_compute(
                        "AllGather", mybir.AluOpType.bypass,
                        replica_groups=[list(range(n_cores))],
                        ins=[cc_in.opt()], outs=[cc_out.opt()])
                    ccv = cc_out.rearrange("(c t p) f -> p c t f", p=P,
                                           c=n_cores)
                    sri, ssum = stats
                    for c8 in range(n_cores):
                        kc0 = c8 * MT + t0
                        nc.sync.dma_start(
                            sri[:, kc0 * 2 * F:(kc0 + NT) * 2 * F]
                            .rearrange("p (t f) -> p t f", t=NT),
                            ccv[:, c8])
                        ssum_add(stats, kc0, kc0 + NT)
                        for t in range(NT):
                            order.append(kc0 + t)

            def wproduct_half(w_sb, b_sb, zs_r, zs_i, dst_r, dst_i, idx, nh):
                """Node-half nh of Y^T = (i * sum_k Z_k W_k + b)^T:
                Yr = -Im(S)+b, Yi = Re(S)+b."""
                for oc in range(FH):
                    s_re = ps.tile([P, HH], f32, tag="prod", bufs=6,
                                   name=f"sre{idx}_{nh}_{oc}")
                    s_im = ps.tile([P, HH], f32, tag="prod", bufs=6,
                                   name=f"sim{idx}_{nh}_{oc}")
                    n_mm = NK * FH
                    cnt = 0
                    for k in range(NK):
                        for fc in range(FH):
                            w_op = w_sb[:, ((fc * NK + k) * FH + oc) * P:
                                        ((fc * NK + k) * FH + oc + 1) * P]
                            zsl = slice(fc * SH + nh * HH,
                                        fc * SH + (nh + 1) * HH)
                            fl = (cnt == 0, cnt == n_mm - 1)
                            nc.tensor.matmul(s_re[:], lhsT=w_op,
                                             rhs=zs_r[k][:, zsl],
                                             start=fl[0], stop=fl[1])
                            nc.tensor.matmul(s_im[:], lhsT=w_op,
                                             rhs=zs_i[k][:, zsl],
                                             start=fl[0], stop=fl[1])
                            cnt += 1
                    osl = slice(oc * SH + nh * HH, oc * SH + (nh + 1) * HH)
                    bia = b_sb[:, oc:oc + 1]
                    with tc.high_priority():
                        nc.scalar.activation(
                            dst_r[:, osl], s_im[:],
                            mybir.ActivationFunctionType.Identity,
                            bias=bia, scale=-1.0)
                        nc.scalar.activation(
                            dst_i[:, osl], s_re[:],
                            mybir.ActivationFunctionType.Identity,
                            bias=bia, scale=1.0)

            # ---- layer 1 ---------------------------------------------------
            st1 = alloc_stationary(0)

            def _load_stat_span(k0, k1):
                sri, ssum = st1
                sl = slice(k0 * 2 * F, k1 * 2 * F)
                nc.sync.dma_start(sri[:, sl], din["xri"][:, sl])
                ssum_add(st1, k0, k1)

            def pre_h0a(kc):
                # chunk 0 alone (earliest possible first matmul), the rest of
                # group 0 next, then one-group-lookahead prefetch so the
                # matmuls never wait on a just-issued group
                if kc == 0:
                    load_l_half(0, 0, 1)
                    _load_stat_span(0, 1)
                elif kc == 1:
                    load_l_half(0, 1, LB)
                    _load_stat_span(1, LB)
                elif kc % LB == 2 and kc + LB - 2 < KC:
                    k0 = kc + LB - 2
                    k1 = min(k0 + LB, KC)
                    load_l_half(0, k0, k1)
                    _load_stat_span(k0, k1)

            def pre_h1a(kc):
                # prefetch node-half B of L during the second A sub-product;
                # the DVE sum-panel adds are deferred past the boundary-1
                # round-0 chain so they can't head-of-line-block it in the
                # in-order DVE queue
                if kc % LB == 0:
                    load_l_half(1, kc, kc + LB, add=False)

            z1t_r = ftp.tile([P, FH * SH], f32r, tag="z1tr", bufs=1, name="z1t_r")
            z1t_i = ftp.tile([P, FH * SH], f32r, tag="z1ti", bufs=1, name="z1t_i")
            ev1 = evict_copy(z1t_r, z1t_i)
            ord1 = list(range(KC))
            sub_product(st1, 0, 0, 0, ord1, ev1, pre_mm=pre_h0a)
            sub_product(st1, 0, 1, 0, ord1, ev1, pre_mm=pre_h1a)

            st2 = alloc_stationary(1)
            ord2 = []
            fire_round(z1t_r, z1t_i, 1, 0, st2, ord2)

            def pre_h0b(kc):
                # the B-half sum-panel adds run here (their DMAs landed during
                # h1A) so they never head-of-line-block the boundary-1 chain
                if kc % LB == 0:
                    lts_add(1, kc, kc + LB)

            sub_product(st1, 0, 0, 1, ord1, ev1, pre_mm=pre_h0b)
            sub_product(st1, 0, 1, 1, ord1, ev1)
            fire_round(z1t_r, z1t_i, 1, 1, st2, ord2)

            # deferred constant loads — complete during product 1 / AG 1
            w1_sb = const.tile([P, FH * NK * FH * P], f32r)
            nc.sync.dma_start(w1_sb[:], din["w1"])
            w2_sb = const.tile([P, FH * NK * FH * P], f32r)
            nc.sync.dma_start(w2_sb[:], din["w2"])
            wc_sb = const.tile([P, 2 * FH * P], f32r)
            nc.sync.dma_start(wc_sb[:], din["wc"])
            b1_sb = const.tile([P, FH], f32)
            nc.sync.dma_start(b1_sb[:], din["b1"])
            b2_sb = const.tile([P, FH], f32)
            nc.sync.dma_start(b2_sb[:], din["b2"])
            bc_sb = const.tile([P, 1], f32)
            nc.sync.dma_start(bc_sb[:], din["bc"])
            x0t_r = ftp.tile([P, FH * SH], f32r, tag="x0tr", bufs=1, name="x0t_r")
            nc.sync.dma_start(x0t_r[:], din["x0tr"])
            x0t_i = ftp.tile([P, FH * SH], f32r, tag="x0ti", bufs=1, name="x0t_i")
            nc.sync.dma_start(x0t_i[:], din["x0ti"])

            # ---- product 2 (Z2 = 2 L Z1 - X), wproduct 1, boundary 2 ------
            z2t_r = ftp.tile([P, FH * SH], f32r, tag="z2tr", bufs=1, name="z2t_r")
            z2t_i = ftp.tile([P, FH * SH], f32r, tag="z2ti", bufs=1, name="z2t_i")
            ev2 = evict_cheb(z2t_r, z2t_i, x0t_r, x0t_i)

            y1t_r = ftp.tile([P, FH * SH], f32r, tag="y1tr", bufs=1, name="y1t_r")
            y1t_i = ftp.tile([P, FH * SH], f32r, tag="y1ti", bufs=1, name="y1t_i")

            st3 = alloc_stationary(2)
            ord3 = []

            sub_product(st2, 1, 0, 0, ord2, ev2)
            sub_product(st2, 1, 1, 0, ord2, ev2)
            wproduct_half(w1_sb, b1_sb, [x0t_r, z1t_r, z2t_r],
                          [x0t_i, z1t_i, z2t_i], y1t_r, y1t_i, 0, 0)
            fire_round(y1t_r, y1t_i, 2, 0, st3, ord3)

            sub_product(st2, 1, 0, 1, ord2, ev2)
            sub_product(st2, 1, 1, 1, ord2, ev2)
            wproduct_half(w1_sb, b1_sb, [x0t_r, z1t_r, z2t_r],
                          [x0t_i, z1t_i, z2t_i], y1t_r, y1t_i, 0, 1)
            fire_round(y1t_r, y1t_i, 2, 1, st3, ord3)

            # ---- product 3 (Z1' = L Y1), boundary 3 ------------------------
            z1pt_r = ftp.tile([P, FH * SH], f32r, tag="z1tr", bufs=1, name="z1pt_r")
            z1pt_i = ftp.tile([P, FH * SH], f32r, tag="z1ti", bufs=1, name="z1pt_i")
            ev3 = evict_copy(z1pt_r, z1pt_i)

            st4 = alloc_stationary(3)
            ord4 = []

            sub_product(st3, 2, 0, 0, ord3, ev3)
            sub_product(st3, 2, 1, 0, ord3, ev3)
            fire_round(z1pt_r, z1pt_i, 3, 0, st4, ord4)

            sub_product(st3, 2, 0, 1, ord3, ev3)
            sub_product(st3, 2, 1, 1, ord3, ev3)
            fire_round(z1pt_r, z1pt_i, 3, 1, st4, ord4)

            # ---- product 4 (Z2' = 2 L Z1' - Y1), wproduct 2, classifier ---
            z2pt_r = ftp.tile([P, FH * SH], f32r, tag="z2tr", bufs=1, name="z2pt_r")
            z2pt_i = ftp.tile([P, FH * SH], f32r, tag="z2ti", bufs=1, name="z2pt_i")
            ev4 = evict_cheb(z2pt_r, z2pt_i, y1t_r, y1t_i)

            y2t_r = ftp.tile([P, FH * SH], f32r, tag="x0tr", bufs=1, name="y2t_r")
            y2t_i = ftp.tile([P, FH * SH], f32r, tag="x0ti", bufs=1, name="y2t_i")

            lg = stg.tile([P, SH], f32r, tag="lg", bufs=1, name="lg")

            def classifier_half(nh):
                # Wc / bc are zero-padded to 128 output classes on host, so
                # the padded logit rows are exactly zero (never read past
                # col C).
                ps_lg = ps.tile([P, HH], f32, tag="aux", bufs=2,
                                name=f"ps_lg{nh}")
                for fcp in range(2 * FH):
                    src = y2t_r if fcp < FH else y2t_i
                    h = fcp % FH
                    nc.tensor.matmul(
                        ps_lg[:], lhsT=wc_sb[:, fcp * P:(fcp + 1) * P],
                        rhs=src[:, h * SH + nh * HH: h * SH + (nh + 1) * HH],
                        start=(fcp == 0), stop=(fcp == 2 * FH - 1))
                nc.scalar.activation(lg[:, nh * HH:(nh + 1) * HH], ps_lg[:],
                                     mybir.ActivationFunctionType.Identity,
                                     bias=bc_sb[:, 0:1], scale=1.0)
                for mt in range(nh * NT, (nh + 1) * NT):
                    tp = ps.tile([P, P], f32r, tag="aux", bufs=2,
                                 name=f"tplg{mt}")
                    nc.tensor.transpose(tp[:], lg[:, mt * P:(mt + 1) * P],
                                        ident[:])
                    lgt = tp[:, 0:C]
                    mneg = sm.tile([P, 1], f32, tag="mneg", bufs=2,
                                   name=f"mneg{mt}")
                    nc.vector.reduce_max(mneg[:], lgt, axis=mybir.AxisListType.X,
                                         negate=True)
                    ex = sm.tile([P, C], f32, tag="ex", bufs=2, name=f"ex{mt}")
                    ssum = sm.tile([P, 1], f32, tag="ssum2", bufs=2,
                                   name=f"ssum{mt}")
                    nc.scalar.activation(ex[:], lgt,
                                         mybir.ActivationFunctionType.Exp,
                                         bias=mneg[:], accum_out=ssum[:])
                    lns = sm.tile([P, 1], f32, tag="lns", bufs=2, name=f"lns{mt}")
                    nc.scalar.activation(lns[:], ssum[:],
                                         mybir.ActivationFunctionType.Ln)
                    ot = sm.tile([P, C], f32, tag="ot", bufs=2, name=f"ot{mt}")
                    nc.vector.tensor_scalar(ot[:], lgt, mneg[:], lns[:],
                                            op0=mybir.AluOpType.add,
                                            op1=mybir.AluOpType.subtract)
                    nc.sync.dma_start(out_d[mt * P:(mt + 1) * P, :], ot[:])

            sub_product(st4, 3, 0, 0, ord4, ev4)
            sub_product(st4, 3, 1, 0, ord4, ev4)
            wproduct_half(w2_sb, b2_sb, [y1t_r, z1pt_r, z2pt_r],
                          [y1t_i, z1pt_i, z2pt_i], y2t_r, y2t_i, 1, 0)
            classifier_half(0)

            sub_product(st4, 3, 0, 1, ord4, ev4)
            sub_product(st4, 3, 1, 1, ord4, ev4)
            wproduct_half(w2_sb, b2_sb, [y1t_r, z1pt_r, z2pt_r],
                          [y1t_i, z1pt_i, z2pt_i], y2t_r, y2t_i, 1, 1)
            classifier_half(1)

    nc.compile()
    return nc


# ---------------------------------------------------------------------------
# Host side: Laplacian assembly + sharding
# ---------------------------------------------------------------------------

def build_lc(edges, q, edge_weight, n):
    """conj(L) of the normalized magnetic Laplacian (max_eigen=2 branch):
    conj(L) = -A_n * exp(-i*Theta).  Returns (Lr, Li) float32 [n, n]."""
    row = np.asarray(edges[0]).astype(np.int64)
    col = np.asarray(edges[1]).astype(np.int64)
    w = np.asarray(edge_weight).astype(np.float32)
    A = np.zeros((n, n), np.float32)
    np.add.at(A, (row, col), w)
    At = A.T.copy()
    A_sym = 0.5 * (A + At)
    d = A_sym.sum(axis=0)
    d[d == 0] = 1.0
    dinv = d ** -0.5
    A_n = (dinv[:, None] * A_sym) * dinv[None, :]
    Theta = (TWO_PI * np.float32(q)) * (A - At)
    Lr = -A_n * np.cos(Theta)
    Li = A_n * np.sin(Theta)
    return Lr.astype(np.float32), Li.astype(np.float32)


def make_in_maps(real, imag, edges, q, edge_weight, W1, b1, W2, b2, Wc, bc,
                 n_nodes=N_NODES, n_cores=N_CORES):
    SH = n_nodes // n_cores
    HH = SH // 2
    real = np.ascontiguousarray(np.asarray(real, dtype=np.float32))
    imag = np.ascontiguousarray(np.asarray(imag, dtype=np.float32))
    KC_ = n_nodes // P

    # node-major [n, F] x2 -> stationary SBUF layout [P, KC*2F] bf16 with
    # r/i interleaved per chunk
    xri = np.concatenate([real.reshape(KC_, P, F), imag.reshape(KC_, P, F)],
                         axis=2).transpose(1, 0, 2).reshape(P, -1)
    xri = np.ascontiguousarray(xri.astype(ml_dtypes.bfloat16))
    Lr, Li = build_lc(np.asarray(edges), float(np.asarray(q)),
                      np.asarray(edge_weight), n_nodes)

    W1 = np.asarray(W1, dtype=np.float32)
    W2 = np.asarray(W2, dtype=np.float32)
    Wc = np.asarray(Wc, dtype=np.float32)
    w1p = np.ascontiguousarray(
        W1.reshape(NK, FH, P, FH, P).transpose(2, 1, 0, 3, 4).reshape(P, -1))
    w2p = np.ascontiguousarray(
        W2.reshape(NK, FH, P, FH, P).transpose(2, 1, 0, 3, 4).reshape(P, -1))
    Wc_pad = np.zeros((P, 2 * F), np.float32)
    Wc_pad[:C, :] = Wc
    wcp = np.ascontiguousarray(
        Wc_pad.T.reshape(2 * FH, P, P).transpose(1, 0, 2).reshape(P, -1))
    b1p = np.ascontiguousarray(
        np.asarray(b1, np.float32).reshape(FH, P).T)
    b2p = np.ascontiguousarray(
        np.asarray(b2, np.float32).reshape(FH, P).T)
    bcp = np.zeros((P, 1), np.float32)
    bcp[:C, 0] = np.asarray(bc, np.float32).reshape(-1)

    in_maps = []
    for c in range(n_cores):
        rows = slice(c * SH, (c + 1) * SH)

        def pack_l(a):
            # Lt [n, SH] -> node-half-major SBUF panel [P, 2*KC*HH] bf16:
            # half-A columns of every chunk first, then half-B
            t = a.reshape(KC_, P, SH).transpose(1, 0, 2)      # [P, KC, SH]
            t = np.concatenate([t[:, :, 0:HH], t[:, :, HH:SH]], axis=1)
            return np.ascontiguousarray(
                t.reshape(P, -1).astype(ml_dtypes.bfloat16))

        ltr = pack_l(Lr[rows, :].T)
        lti = pack_l(Li[rows, :].T)
        x0tr = np.ascontiguousarray(
            real[rows, :].T.reshape(FH, P, SH).transpose(1, 0, 2).reshape(P, -1))
        x0ti = np.ascontiguousarray(
            imag[rows, :].T.reshape(FH, P, SH).transpose(1, 0, 2).reshape(P, -1))
        in_maps.append({
            "ltr": ltr, "lti": lti,
            "xri": xri,
            "x0tr": x0tr, "x0ti": x0ti,
            "w1": w1p, "w2": w2p, "wc": wcp,
            "b1": b1p, "b2": b2p, "bc": bcp,
        })
    return in_maps


_NC_CACHE = {}


def _get_nc():
    if "nc" not in _NC_CACHE:
        _NC_CACHE["nc"] = build_nc()
    return _NC_CACHE["nc"]


def kernel(real, imag, edges, q, edge_weight, W1, b1, W2, b2, Wc, bc,
           _run_kwargs=None):
    in_maps = make_in_maps(real, imag, edges, q, edge_weight,
                           W1, b1, W2, b2, Wc, bc)
    nc = _get_nc()
    res = bass_utils.run_bass_kernel_spmd(
        nc, in_maps, core_ids=list(range(N_CORES)), **(_run_kwargs or {}))
    out = np.concatenate([res.results[c]["out"] for c in range(N_CORES)], axis=0)
    if _run_kwargs:
        _NC_CACHE["last_result"] = res
    return out


# revision 19
# speedup vs baseline: 1.0667x; 1.0667x over previous
"""ChebNet (magnetic-Laplacian ChebConv, K=2, 2 layers + linear classifier +
log_softmax) on 8 Trainium2 NeuronCores.

Strategy: 1D row-shard of the (dense) conjugated magnetic Laplacian Lc across
8 cores (512 rows each).  The Laplacian is assembled on host from the edge
list; all matmuls, Chebyshev recursion, biases, classifier and log_softmax
run on device.

Pipelined-boundary version: every product is split into 4 sub-products
(feature-half x node-half) with separate PSUM banks so the node-half-A
output can be evicted and AllGather'ed while node-half-B is still being
computed.  Consumers process gathered chunks in delivery order and close
their own A half first, so each boundary's round-0 collective overlaps
compute on both sides.  The Karatsuba sum panels (L_r+L_i and X_r+X_i) are
computed on device (DVE) instead of being shipped from HBM, and the L
panels are packed node-half-major so half A streams from HBM first.
"""

import sys

for _p in ("/opt/trn_rl_repo",):
    if _p not in sys.path:
        sys.path.insert(0, _p)

import numpy as np
import ml_dtypes

import concourse.bass as bass
import concourse.mybir as mybir
import concourse.tile as tile
from concourse import bacc
from concourse import bass_utils
from concourse.masks import make_identity

P = 128          # partitions
F = 256          # feature width of X / hidden layers
FH = F // P      # feature halves (2)
NK = 3           # Chebyshev orders (K+1)
C = 40           # classes
N_NODES = 4096
N_CORES = 8
TWO_PI = 2.0 * np.pi

f32 = mybir.dt.float32
f32r = mybir.dt.float32r
bf16 = mybir.dt.bfloat16


# ---------------------------------------------------------------------------
# Device program
# ---------------------------------------------------------------------------

def build_nc(n_nodes=N_NODES, n_cores=N_CORES):
    KC = n_nodes // P            # contraction chunks (32)
    SH = n_nodes // n_cores      # local rows per core (512)
    HH = SH // 2                 # node half (256)
    MT = SH // P                 # local row tiles (4)
    NT = HH // P                 # row tiles per half (2)
    LB = 4                       # L chunks per load DMA group

    nc = bacc.Bacc("TRN2", target_bir_lowering=False, debug=False,
                   num_devices=n_cores)

    din = {}
    for nm, shp, dt in [
        # L^T panels, node-half-major: [half][kc][HH]
        ("ltr", [P, KC * SH], bf16),
        ("lti", [P, KC * SH], bf16),
        # stationary X, node-major chunks, r/i interleaved per chunk
        ("xri", [P, KC * 2 * F], bf16),
        ("x0tr", [P, FH * SH], f32r), ("x0ti", [P, FH * SH], f32r),
        ("w1", [P, FH * NK * FH * P], f32r), ("w2", [P, FH * NK * FH * P], f32r),
        ("wc", [P, 2 * FH * P], f32r),
        ("b1", [P, FH], f32), ("b2", [P, FH], f32), ("bc", [P, 1], f32),
    ]:
        din[nm] = nc.dram_tensor(nm, shp, dt, kind="ExternalInput").ap()
    out_d = nc.dram_tensor("out", [SH, C], f32, kind="ExternalOutput").ap()

    with tile.TileContext(nc) as tc:
        with (
            tc.tile_pool(name="const", bufs=1) as const,
            tc.tile_pool(name="lres", bufs=1) as lres,
            tc.tile_pool(name="stat", bufs=1) as stat,
            tc.tile_pool(name="ftp", bufs=1) as ftp,
            tc.tile_pool(name="stg", bufs=1) as stg,
            tc.tile_pool(name="sm", bufs=2) as sm,
            tc.tile_pool(name="ps", bufs=1, space="PSUM") as ps,
            tc.tile_pool(name="dram", bufs=1, space="DRAM") as dram,
        ):
            # ---- resident Laplacian panels (ltr/lti from HBM once; the sum
            # panel is computed on DVE) ---------------------------------------
            ltr_sb = lres.tile([P, KC * SH], bf16, tag="ltr", bufs=1, name="ltr_sb")
            lti_sb = lres.tile([P, KC * SH], bf16, tag="lti", bufs=1, name="lti_sb")
            lts_sb = lres.tile([P, KC * SH], bf16, tag="lts", bufs=1, name="lts_sb")

            def load_l_half(half, k0, k1, add=True):
                """Load ltr/lti chunks [k0,k1) of node-half `half`; optionally
                compute the Karatsuba sum panel for the same span."""
                sl = slice(half * KC * HH + k0 * HH, half * KC * HH + k1 * HH)
                nc.sync.dma_start(ltr_sb[:, sl], din["ltr"][:, sl])
                nc.sync.dma_start(lti_sb[:, sl], din["lti"][:, sl])
                if add:
                    lts_add(half, k0, k1)

            def lts_add(half, k0, k1):
                sl = slice(half * KC * HH + k0 * HH, half * KC * HH + k1 * HH)
                nc.vector.tensor_add(lts_sb[:, sl], ltr_sb[:, sl], lti_sb[:, sl])

            # ---- identity (no HBM traffic; needed by first boundary) -------
            ident_f = const.tile([P, P], f32)
            make_identity(nc, ident_f[:])
            ident = const.tile([P, P], f32r)
            nc.vector.tensor_copy(ident[:], ident_f[:])

            # ---- helpers ---------------------------------------------------
            def alloc_stationary(idx):
                sri = stat.tile([P, KC * 2 * F], bf16, tag="sri", bufs=1,
                                name=f"sri{idx}")
                ssum = stat.tile([P, KC * F], bf16, tag="ssum", bufs=1,
                                 name=f"ssum{idx}")
                return sri, ssum

            def ssum_add(stats, k0, k1):
                """ssum[k0:k1) = r + i from the interleaved sri layout."""
                sri, ssum = stats
                sv = sri.rearrange("p (k f) -> p k f", f=2 * F)
                dv = ssum.rearrange("p (k f) -> p k f", f=F)
                nc.vector.tensor_add(dv[:, k0:k1], sv[:, k0:k1, 0:F],
                                     sv[:, k0:k1, F:2 * F])

            def sub_product(stats, idx, h, half, order, evict, pre_mm=None):
                """One (feature-half h, node-half half) Karatsuba sub-product:
                P1 = Lr@Sr, P2 = Li@Si, P3 = (Lr+Li)@(Sr+Si) over `order`
                chunks, each [P, HH] in its own PSUM bank.  evict(p1,p2,p3)
                combines them into the destination half."""
                sri, ssum = stats
                nm = f"{idx}_{h}_{half}"
                p1 = ps.tile([P, HH], f32, tag="prod", bufs=6, name=f"p1_{nm}")
                p2 = ps.tile([P, HH], f32, tag="prod", bufs=6, name=f"p2_{nm}")
                p3 = ps.tile([P, HH], f32, tag="prod", bufs=6, name=f"p3_{nm}")
                base = half * KC * HH
                for j, kc in enumerate(order):
                    if pre_mm is not None:
                        pre_mm(kc)
                    lr = ltr_sb[:, base + kc * HH: base + (kc + 1) * HH]
                    li = lti_sb[:, base + kc * HH: base + (kc + 1) * HH]
                    ls = lts_sb[:, base + kc * HH: base + (kc + 1) * HH]
                    o_r = kc * 2 * F + h * P
                    o_i = kc * 2 * F + F + h * P
                    o_s = kc * F + h * P
                    first, last = j == 0, j == len(order) - 1
                    nc.tensor.matmul(p1[:], lhsT=sri[:, o_r:o_r + P], rhs=lr,
                                     start=first, stop=last)
                    nc.tensor.matmul(p2[:], lhsT=sri[:, o_i:o_i + P], rhs=li,
                                     start=first, stop=last)
                    nc.tensor.matmul(p3[:], lhsT=ssum[:, o_s:o_s + P], rhs=ls,
                                     start=first, stop=last)
                evict(p1, p2, p3, h, half)

            # DVE may read at most ONE PSUM operand per op: bounce P2
            # through SBUF scratch, then combine against P1/P3.
            def evict_copy(dst_r, dst_i):
                def fn(p1, p2, p3, h, half):
                    sl = slice(h * SH + half * HH, h * SH + (half + 1) * HH)
                    with tc.high_priority():
                        t2 = stg.tile([P, HH], f32, tag="scr", bufs=2,
                                      name=f"t2c{id(dst_r)}_{h}_{half}")
                        nc.vector.tensor_copy(t2[:], p2[:])
                        nc.vector.tensor_sub(dst_r[:, sl], p1[:], t2[:])
                        nc.vector.tensor_sub(dst_i[:, sl], p3[:], t2[:])
                        nc.vector.tensor_sub(dst_i[:, sl], dst_i[:, sl], p1[:])
                return fn

            def evict_cheb(dst_r, dst_i, z0_r, z0_i):
                """dst = 2*Z - z0 (Chebyshev T2 step), fused eviction."""
                def fn(p1, p2, p3, h, half):
                    sl = slice(h * SH + half * HH, h * SH + (half + 1) * HH)
                    with tc.high_priority():
                        t2 = stg.tile([P, HH], f32, tag="scr", bufs=2,
                                      name=f"t2x{id(dst_r)}_{h}_{half}")
                        u = stg.tile([P, HH], f32, tag="scr2", bufs=2,
                                     name=f"u{id(dst_r)}_{h}_{half}")
                        nc.vector.tensor_copy(t2[:], p2[:])
                        nc.vector.tensor_sub(u[:], p1[:], t2[:])
                        nc.vector.scalar_tensor_tensor(
                            dst_r[:, sl], u[:], 2.0, z0_r[:, sl],
                            op0=mybir.AluOpType.mult,
                            op1=mybir.AluOpType.subtract)
                        nc.vector.tensor_sub(u[:], p3[:], t2[:])
                        nc.vector.tensor_sub(u[:], u[:], p1[:])
                        nc.vector.scalar_tensor_tensor(
                            dst_i[:, sl], u[:], 2.0, z0_i[:, sl],
                            op0=mybir.AluOpType.mult,
                            op1=mybir.AluOpType.subtract)
                return fn

            def fire_round(src_r, src_i, idx, ri, stats, order):
                """AG one node-half round: PE-transpose the half's local Z^T
                to node-major bf16, sub-AllGather, reload the delivered global
                chunks into `stats`, extend `order` with the delivered chunk
                ids."""
                t0 = ri * NT
                with tc.high_priority():
                    stage = stg.tile([P, MT * 2 * F], bf16, tag="stage", bufs=1,
                                     name=f"stage{idx}")
                    for mt in range(t0, t0 + NT):
                        for ci, src in enumerate((src_r, src_i)):
                            for h in range(FH):
                                tp = ps.tile([P, P], f32r, tag="aux", bufs=2,
                                             name=f"tp{idx}_{mt}_{ci}_{h}")
                                nc.tensor.transpose(
                                    tp[:],
                                    src[:, h * SH + mt * P:
                                        h * SH + (mt + 1) * P],
                                    ident[:])
                                dst = stage[:, mt * 2 * F + ci * F + h * P:
                                            mt * 2 * F + ci * F + (h + 1) * P]
                                nc.vector.tensor_copy(dst, tp[:])
                    cc_in = dram.tile([NT * P, 2 * F], bf16, tag=f"ccin{ri}",
                                      bufs=2, name=f"ccin{idx}_{ri}")
                    cc_out = dram.tile([n_cores * NT * P, 2 * F], bf16,
                                       tag=f"ccout{ri}", bufs=2,
                                       name=f"ccout{idx}_{ri}",
                                       addr_space="Shared")
                    nc.sync.dma_start(
                        cc_in.rearrange("(t p) f -> p t f", p=P),
                        stage.rearrange("p (mt f) -> p mt f", mt=MT)
                             [:, t0:t0 + NT])
                    nc.gpsimd.collective_compute(
                        "AllGather", mybir.AluOpType.bypass,
                        replica_groups=[list(range(n_cores))],
                        ins=[cc_in.opt()], outs=[cc_out.opt()])
                    ccv = cc_out.rearrange("(c t p) f -> p c t f", p=P,
                                           c=n_cores)
                    sri, ssum = stats
                    for c8 in range(n_cores):
                        kc0 = c8 * MT + t0
                        nc.sync.dma_start(
                            sri[:, kc0 * 2 * F:(kc0 + NT) * 2 * F]
                            .rearrange("p (t f) -> p t f", t=NT),
                            ccv[:, c8])
                        ssum_add(stats, kc0, kc0 + NT)
                        for t in range(NT):
                            order.append(kc0 + t)

            def wproduct_half(w_sb, b_sb, zs_r, zs_i, dst_r, dst_i, idx, nh):
                """Node-half nh of Y^T = (i * sum_k Z_k W_k + b)^T:
                Yr = -Im(S)+b, Yi = Re(S)+b."""
                for oc in range(FH):
                    s_re = ps.tile([P, HH], f32, tag="prod", bufs=6,
                                   name=f"sre{idx}_{nh}_{oc}")
                    s_im = ps.tile([P, HH], f32, tag="prod", bufs=6,
                                   name=f"sim{idx}_{nh}_{oc}")
                    n_mm = NK * FH
                    cnt = 0
                    for k in range(NK):
                        for fc in range(FH):
                            w_op = w_sb[:, ((fc * NK + k) * FH + oc) * P:
                                        ((fc * NK + k) * FH + oc + 1) * P]
                            zsl = slice(fc * SH + nh * HH,
                                        fc * SH + (nh + 1) * HH)
                            fl = (cnt == 0, cnt == n_mm - 1)
                            nc.tensor.matmul(s_re[:], lhsT=w_op,
                                             rhs=zs_r[k][:, zsl],
                                             start=fl[0], stop=fl[1])
                            nc.tensor.matmul(s_im[:], lhsT=w_op,
                                             rhs=zs_i[k][:, zsl],
                                             start=fl[0], stop=fl[1])
                            cnt += 1
                    osl = slice(oc * SH + nh * HH, oc * SH + (nh + 1) * HH)
                    bia = b_sb[:, oc:oc + 1]
                    with tc.high_priority():
                        nc.scalar.activation(
                            dst_r[:, osl], s_im[:],
                            mybir.ActivationFunctionType.Identity,
                            bias=bia, scale=-1.0)
                        nc.scalar.activation(
                            dst_i[:, osl], s_re[:],
                            mybir.ActivationFunctionType.Identity,
                            bias=bia, scale=1.0)

            # ---- layer 1 ---------------------------------------------------
            st1 = alloc_stationary(0)

            def _load_stat_span(k0, k1):
                sri, ssum = st1
                sl = slice(k0 * 2 * F, k1 * 2 * F)
                nc.sync.dma_start(sri[:, sl], din["xri"][:, sl])
                ssum_add(st1, k0, k1)

            def pre_h0a(kc):
                # chunk 0 alone (earliest possible first matmul), the rest of
                # group 0 next, then one-group-lookahead prefetch so the
                # matmuls never wait on a just-issued group
                if kc == 0:
                    load_l_half(0, 0, 1)
                    _load_stat_span(0, 1)
                elif kc == 1:
                    load_l_half(0, 1, LB)
                    _load_stat_span(1, LB)
                elif kc % LB == 2 and kc + LB - 2 < KC:
                    k0 = kc + LB - 2
                    k1 = min(k0 + LB, KC)
                    load_l_half(0, k0, k1)
                    _load_stat_span(k0, k1)

            def pre_h1a(kc):
                # prefetch node-half B of L during the second A sub-product;
                # the DVE sum-panel adds are deferred past the boundary-1
                # round-0 chain so they can't head-of-line-block it in the
                # in-order DVE queue
                if kc % LB == 0:
                    load_l_half(1, kc, kc + LB, add=False)

            z1t_r = ftp.tile([P, FH * SH], f32r, tag="z1tr", bufs=1, name="z1t_r")
            z1t_i = ftp.tile([P, FH * SH], f32r, tag="z1ti", bufs=1, name="z1t_i")
            ev1 = evict_copy(z1t_r, z1t_i)
            ord1 = list(range(KC))
            sub_product(st1, 0, 0, 0, ord1, ev1, pre_mm=pre_h0a)
            sub_product(st1, 0, 1, 0, ord1, ev1, pre_mm=pre_h1a)

            st2 = alloc_stationary(1)
            ord2 = []
            fire_round(z1t_r, z1t_i, 1, 0, st2, ord2)

            def pre_h0b(kc):
                # the B-half sum-panel adds run here (their DMAs landed during
                # h1A) so they never head-of-line-block the boundary-1 chain
                if kc % LB == 0:
                    lts_add(1, kc, kc + LB)

            sub_product(st1, 0, 0, 1, ord1, ev1, pre_mm=pre_h0b)
            sub_product(st1, 0, 1, 1, ord1, ev1)
            fire_round(z1t_r, z1t_i, 1, 1, st2, ord2)

            # deferred constant loads — complete during product 1 / AG 1
            w1_sb = const.tile([P, FH * NK * FH * P], f32r)
            nc.sync.dma_start(w1_sb[:], din["w1"])
            w2_sb = const.tile([P, FH * NK * FH * P], f32r)
            nc.sync.dma_start(w2_sb[:], din["w2"])
            wc_sb = const.tile([P, 2 * FH * P], f32r)
            nc.sync.dma_start(wc_sb[:], din["wc"])
            b1_sb = const.tile([P, FH], f32)
            nc.sync.dma_start(b1_sb[:], din["b1"])
            b2_sb = const.tile([P, FH], f32)
            nc.sync.dma_start(b2_sb[:], din["b2"])
            bc_sb = const.tile([P, 1], f32)
            nc.sync.dma_start(bc_sb[:], din["bc"])
            x0t_r = ftp.tile([P, FH * SH], f32r, tag="x0tr", bufs=1, name="x0t_r")
            nc.sync.dma_start(x0t_r[:], din["x0tr"])
            x0t_i = ftp.tile([P, FH * SH], f32r, tag="x0ti", bufs=1, name="x0t_i")
            nc.sync.dma_start(x0t_i[:], din["x0ti"])

            # ---- product 2 (Z2 = 2 L Z1 - X), wproduct 1, boundary 2 ------
            z2t_r = ftp.tile([P, FH * SH], f32r, tag="z2tr", bufs=1, name="z2t_r")
            z2t_i = ftp.tile([P, FH * SH], f32r, tag="z2ti", bufs=1, name="z2t_i")
            ev2 = evict_cheb(z2t_r, z2t_i, x0t_r, x0t_i)

            y1t_r = ftp.tile([P, FH * SH], f32r, tag="y1tr", bufs=1, name="y1t_r")
            y1t_i = ftp.tile([P, FH * SH], f32r, tag="y1ti", bufs=1, name="y1t_i")

            st3 = alloc_stationary(2)
            ord3 = []

            sub_product(st2, 1, 0, 0, ord2, ev2)
            sub_product(st2, 1, 1, 0, ord2, ev2)
            wproduct_half(w1_sb, b1_sb, [x0t_r, z1t_r, z2t_r],
                          [x0t_i, z1t_i, z2t_i], y1t_r, y1t_i, 0, 0)
            fire_round(y1t_r, y1t_i, 2, 0, st3, ord3)

            sub_product(st2, 1, 0, 1, ord2, ev2)
            sub_product(st2, 1, 1, 1, ord2, ev2)
            wproduct_half(w1_sb, b1_sb, [x0t_r, z1t_r, z2t_r],
                          [x0t_i, z1t_i, z2t_i], y1t_r, y1t_i, 0, 1)
            fire_round(y1t_r, y1t_i, 2, 1, st3, ord3)

            # ---- product 3 (Z1' = L Y1), boundary 3 ------------------------
            z1pt_r = ftp.tile([P, FH * SH], f32r, tag="z1tr", bufs=1, name="z1pt_r")
            z1pt_i = ftp.tile([P, FH * SH], f32r, tag="z1ti", bufs=1, name="z1pt_i")
            ev3 = evict_copy(z1pt_r, z1pt_i)

            st4 = alloc_stationary(3)
            ord4 = []

            sub_product(st3, 2, 0, 0, ord3, ev3)
            sub_product(st3, 2, 1, 0, ord3, ev3)
            fire_round(z1pt_r, z1pt_i, 3, 0, st4, ord4)

            sub_product(st3, 2, 0, 1, ord3, ev3)
            sub_product(st3, 2, 1, 1, ord3, ev3)
            fire_round(z1pt_r, z1pt_i, 3, 1, st4, ord4)

            # ---- product 4 (Z2' = 2 L Z1' - Y1), wproduct 2, classifier ---
            z2pt_r = ftp.tile([P, FH * SH], f32r, tag="z2tr", bufs=1, name="z2pt_r")
            z2pt_i = ftp.tile([P, FH * SH], f32r, tag="z2ti", bufs=1, name="z2pt_i")
            ev4 = evict_cheb(z2pt_r, z2pt_i, y1t_r, y1t_i)

            y2t_r = ftp.tile([P, FH * SH], f32r, tag="x0tr", bufs=1, name="y2t_r")
            y2t_i = ftp.tile([P, FH * SH], f32r, tag="x0ti", bufs=1, name="y2t_i")

            lg = stg.tile([P, SH], f32r, tag="lg", bufs=1, name="lg")

            def classifier_half(nh):
                # Wc / bc are zero-padded to 128 output classes on host, so
                # the padded logit rows are exactly zero (never read past
                # col C).
                ps_lg = ps.tile([P, HH], f32, tag="aux", bufs=2,
                                name=f"ps_lg{nh}")
                for fcp in range(2 * FH):
                    src = y2t_r if fcp < FH else y2t_i
                    h = fcp % FH
                    nc.tensor.matmul(
                        ps_lg[:], lhsT=wc_sb[:, fcp * P:(fcp + 1) * P],
                        rhs=src[:, h * SH + nh * HH: h * SH + (nh + 1) * HH],
                        start=(fcp == 0), stop=(fcp == 2 * FH - 1))
                nc.scalar.activation(lg[:, nh * HH:(nh + 1) * HH], ps_lg[:],
                                     mybir.ActivationFunctionType.Identity,
                                     bias=bc_sb[:, 0:1], scale=1.0)
                for mt in range(nh * NT, (nh + 1) * NT):
                    tp = ps.tile([P, P], f32r, tag="aux", bufs=2,
                                 name=f"tplg{mt}")
                    nc.tensor.transpose(tp[:], lg[:, mt * P:(mt + 1) * P],
                                        ident[:])
                    lgt = tp[:, 0:C]
                    mneg = sm.tile([P, 1], f32, tag="mneg", bufs=2,
                                   name=f"mneg{mt}")
                    nc.vector.reduce_max(mneg[:], lgt, axis=mybir.AxisListType.X,
                                         negate=True)
                    ex = sm.tile([P, C], f32, tag="ex", bufs=2, name=f"ex{mt}")
                    ssum = sm.tile([P, 1], f32, tag="ssum2", bufs=2,
                                   name=f"ssum{mt}")
                    nc.scalar.activation(ex[:], lgt,
                                         mybir.ActivationFunctionType.Exp,
                                         bias=mneg[:], accum_out=ssum[:])
                    lns = sm.tile([P, 1], f32, tag="lns", bufs=2, name=f"lns{mt}")
                    nc.scalar.activation(lns[:], ssum[:],
                                         mybir.ActivationFunctionType.Ln)
                    ot = sm.tile([P, C], f32, tag="ot", bufs=2, name=f"ot{mt}")
                    nc.vector.tensor_scalar(ot[:], lgt, mneg[:], lns[:],
                                            op0=mybir.AluOpType.add,
                                            op1=mybir.AluOpType.subtract)
                    nc.sync.dma_start(out_d[mt * P:(mt + 1) * P, :], ot[:])

            sub_product(st4, 3, 0, 0, ord4, ev4)
            sub_product(st4, 3, 1, 0, ord4, ev4)
            wproduct_half(w2_sb, b2_sb, [y1t_r, z1pt_r, z2pt_r],
                          [y1t_i, z1pt_i, z2pt_i], y2t_r, y2t_i, 1, 0)
            classifier_half(0)

            sub_product(st4, 3, 0, 1, ord4, ev4)
            sub_product(st4, 3, 1, 1, ord4, ev4)
            wproduct_half(w2_sb, b2_sb, [y1t_r, z1pt_r, z2pt_r],
                          [y1t_i, z1pt_i, z2pt_i], y2t_r, y2t_i, 1, 1)
            classifier_half(1)

    nc.compile()
    return nc


# ---------------------------------------------------------------------------
# Host side: Laplacian assembly + sharding
# ---------------------------------------------------------------------------

def build_lc(edges, q, edge_weight, n):
    """conj(L) of the normalized magnetic Laplacian (max_eigen=2 branch):
    conj(L) = -A_n * exp(-i*Theta).  Returns (Lr, Li) float32 [n, n]."""
    row = np.asarray(edges[0]).astype(np.int64)
    col = np.asarray(edges[1]).astype(np.int64)
    w = np.asarray(edge_weight).astype(np.float32)
    A = np.zeros((n, n), np.float32)
    np.add.at(A, (row, col), w)
    At = A.T.copy()
    A_sym = 0.5 * (A + At)
    d = A_sym.sum(axis=0)
    d[d == 0] = 1.0
    dinv = d ** -0.5
    A_n = (dinv[:, None] * A_sym) * dinv[None, :]
    Theta = (TWO_PI * np.float32(q)) * (A - At)
    Lr = -A_n * np.cos(Theta)
    Li = A_n * np.sin(Theta)
    return Lr.astype(np.float32), Li.astype(np.float32)


def make_in_maps(real, imag, edges, q, edge_weight, W1, b1, W2, b2, Wc, bc,
                 n_nodes=N_NODES, n_cores=N_CORES):
    SH = n_nodes // n_cores
    HH = SH // 2
    real = np.ascontiguousarray(np.asarray(real, dtype=np.float32))
    imag = np.ascontiguousarray(np.asarray(imag, dtype=np.float32))
    KC_ = n_nodes // P

    # node-major [n, F] x2 -> stationary SBUF layout [P, KC*2F] bf16 with
    # r/i interleaved per chunk
    xri = np.concatenate([real.reshape(KC_, P, F), imag.reshape(KC_, P, F)],
                         axis=2).transpose(1, 0, 2).reshape(P, -1)
    xri = np.ascontiguousarray(xri.astype(ml_dtypes.bfloat16))
    Lr, Li = build_lc(np.asarray(edges), float(np.asarray(q)),
                      np.asarray(edge_weight), n_nodes)

    W1 = np.asarray(W1, dtype=np.float32)
    W2 = np.asarray(W2, dtype=np.float32)
    Wc = np.asarray(Wc, dtype=np.float32)
    w1p = np.ascontiguousarray(
        W1.reshape(NK, FH, P, FH, P).transpose(2, 1, 0, 3, 4).reshape(P, -1))
    w2p = np.ascontiguousarray(
        W2.reshape(NK, FH, P, FH, P).transpose(2, 1, 0, 3, 4).reshape(P, -1))
    Wc_pad = np.zeros((P, 2 * F), np.float32)
    Wc_pad[:C, :] = Wc
    wcp = np.ascontiguousarray(
        Wc_pad.T.reshape(2 * FH, P, P).transpose(1, 0, 2).reshape(P, -1))
    b1p = np.ascontiguousarray(
        np.asarray(b1, np.float32).reshape(FH, P).T)
    b2p = np.ascontiguousarray(
        np.asarray(b2, np.float32).reshape(FH, P).T)
    bcp = np.zeros((P, 1), np.float32)
    bcp[:C, 0] = np.asarray(bc, np.float32).reshape(-1)

    in_maps = []
    for c in range(n_cores):
        rows = slice(c * SH, (c + 1) * SH)

        def pack_l(a):
            # Lt [n, SH] -> node-half-major SBUF panel [P, 2*KC*HH] bf16:
            # half-A columns of every chunk first, then half-B
            t = a.reshape(KC_, P, SH).transpose(1, 0, 2)      # [P, KC, SH]
            t = np.concatenate([t[:, :, 0:HH], t[:, :, HH:SH]], axis=1)
            return np.ascontiguousarray(
                t.reshape(P, -1).astype(ml_dtypes.bfloat16))

        ltr = pack_l(Lr[rows, :].T)
        lti = pack_l(Li[rows, :].T)
        x0tr = np.ascontiguousarray(
            real[rows, :].T.reshape(FH, P, SH).transpose(1, 0, 2).reshape(P, -1))
        x0ti = np.ascontiguousarray(
            imag[rows, :].T.reshape(FH, P, SH).transpose(1, 0, 2).reshape(P, -1))
        in_maps.append({
            "ltr": ltr, "lti": lti,
            "xri": xri,
            "x0tr": x0tr, "x0ti": x0ti,
            "w1": w1p, "w2": w2p, "wc": wcp,
            "b1": b1p, "b2": b2p, "bc": bcp,
        })
    return in_maps


_NC_CACHE = {}


def _get_nc():
    if "nc" not in _NC_CACHE:
        _NC_CACHE["nc"] = build_nc()
    return _NC_CACHE["nc"]


def kernel(real, imag, edges, q, edge_weight, W1, b1, W2, b2, Wc, bc,
           _run_kwargs=None):
    in_maps = make_in_maps(real, imag, edges, q, edge_weight,
                           W1, b1, W2, b2, Wc, bc)
    nc = _get_nc()
    res = bass_utils.run_bass_kernel_spmd(
        nc, in_maps, core_ids=list(range(N_CORES)), **(_run_kwargs or {}))
    out = np.concatenate([res.results[c]["out"] for c in range(N_CORES)], axis=0)
    if _run_kwargs:
        _NC_CACHE["last_result"] = res
    return out
